# revision 1
# baseline (speedup 1.0000x reference)
"""Trainium2 Bass kernel for nn_MDLoss (retrieval_knn).

reference:
    distance[b, g, p] = ||ini_pred[b, p] - gt[b, g]||^2
    index_gt = argmin_g distance          -> [B, Np]
    gt_matched = gt[b, index_gt]          -> [B, Np, 2]
    loss = |pred - gt_matched|.mean()

Strategy (pure data-parallel over B across 8 cores, 32 instances each):
  - scores s[p, g] = 2*px*gx + 2*py*gy - (gx^2+gy^2); argmax_g s == argmin_g dist.
    Computed on the PE as a k=11 matmul of bf16 hi/lo-split operands (full PE
    rate, ~1e-6 absolute accuracy -> ~2e-6 relative error on the final mean).
       lhsT rows: [phx, phx, plx, plx, phy, phy, ply, ply, 1, 1, 1]
       rhs  rows: [Ghx, Glx, Ghx, Glx, Ghy, Gly, Ghy, Gly, R2h, R2m, R2l]
  - argmax per query via DVE max8 + max_index on the PSUM score tile.
  - gt gather via SWDGE indirect DMA (one offset per partition per call).
  - |pred - gt*| via DVE sub + ACT Abs with accumulate; partition reduce via
    a ones-matmul; per-core sum returned, combined on host in float64.

Layout: 512 queries/instance as 4 tiles of 128 partitions; position t*128+p of
the P rows holds query q = 4p+t, so per-tile argmax indices land in column t
of a [128, 4] offset tile matching a contiguous pred layout [128, (t, c)].
"""
import sys
import numpy as np

sys.path.insert(0, "/opt/trn_rl_repo")

import ml_dtypes  # noqa: E402
import concourse.bass as bass  # noqa: E402
import concourse.bacc as bacc  # noqa: E402
import concourse.tile as tile  # noqa: E402
from concourse import mybir  # noqa: E402
from concourse import bass_utils  # noqa: E402

B, NP_, NG, D = 256, 512, 1024, 2
NCORES = 8
NI = B // NCORES          # 32 instances per core
NT = NP_ // 128           # 4 query tiles per instance

f32 = mybir.dt.float32
bf16 = mybir.dt.bfloat16
u32 = mybir.dt.uint32
i32 = mybir.dt.int32
Sq = mybir.ActivationFunctionType.Square
Abs = mybir.ActivationFunctionType.Abs
SQH = 0.7071067811865476  # sqrt(1/2): Square(g*SQH) = g^2/2


def _build(nc):
    Pd = nc.dram_tensor("Pd", [2, NI, NP_], f32, kind="ExternalInput")
    Gxyd = nc.dram_tensor("Gxyd", [2, NI, NG], f32, kind="ExternalInput")
    ONESd = nc.dram_tensor("ONESd", [3, NI, NP_], bf16, kind="ExternalInput")
    GTd = nc.dram_tensor("GTd", [NI * NG, 2], f32, kind="ExternalInput")
    PRd = nc.dram_tensor("PRd", [NI, 128, NT * 2], f32, kind="ExternalInput")
    LOSSd = nc.dram_tensor("LOSSd", [1, 1], f32, kind="ExternalOutput")

    with tile.TileContext(nc) as tc:
        with (
            tc.tile_pool(name="sb", bufs=1) as sb,
            tc.tile_pool(name="sc", bufs=3) as sc,
            tc.tile_pool(name="ps", bufs=3, space="PSUM") as ps,
            tc.tile_pool(name="ps1", bufs=1, space="PSUM") as ps1,
        ):
            # ---------- G side: [2gx, 2gy, -(gx^2+gy^2)] + bf16 hi/lo split ----------
            Gsp = sb.tile([32, 3, NG], f32)
            nc.sync.dma_start(Gsp[:, 0:2, :], Gxyd[:].rearrange("c b g -> b c g"))
            Gsq = sb.tile([32, 2, NG], f32)
            nc.scalar.activation(Gsq[:], Gsp[:, 0:2, :], Sq, scale=SQH)  # g^2/2
            nc.vector.scalar_tensor_tensor(
                out=Gsp[:, 2, :], in0=Gsq[:, 0, :], scalar=-1.0, in1=Gsq[:, 1, :],
                op0=mybir.AluOpType.mult, op1=mybir.AluOpType.subtract,
            )  # -(gx^2+gy^2)/2
            nc.scalar.mul(Gsp[:], Gsp[:], 2.0)  # single final producer of Gsp

            Gh = sb.tile([32, 3, NG], bf16)
            nc.vector.tensor_copy(Gh[:], Gsp[:])
            T1 = sb.tile([32, 3, NG], f32)
            nc.vector.tensor_sub(T1[:], Gsp[:], Gh[:])
            Gl = sb.tile([32, 3, NG], bf16)
            nc.vector.tensor_copy(Gl[:], T1[:])
            R2l = sb.tile([32, NG], bf16)
            nc.vector.tensor_sub(R2l[:], T1[:, 2, :], Gl[:, 2, :])

            Grhs = sb.tile([11, NI, NG], bf16)
            for r, srct, c in [(0, Gh, 0), (1, Gl, 0), (2, Gh, 0), (3, Gl, 0),
                               (4, Gh, 1), (5, Gl, 1), (6, Gh, 1), (7, Gl, 1),
                               (8, Gh, 2), (9, Gl, 2)]:
                nc.gpsimd.dma_start(Grhs[r:r + 1, :, :], srct[:, c, :])
            nc.gpsimd.dma_start(Grhs[10:11, :, :], R2l[:, :])

            # ---------- P side: bf16 hi/lo split ----------
            Psp = sb.tile([32, 2, NP_], f32)
            nc.sync.dma_start(Psp[:], Pd[:].rearrange("c b q -> b c q"))
            Ph = sb.tile([32, 2, NP_], bf16)
            nc.vector.tensor_copy(Ph[:], Psp[:])
            Pl = sb.tile([32, 2, NP_], bf16)
            nc.vector.tensor_sub(Pl[:], Psp[:], Ph[:])

            Plhs = sb.tile([11, NI, NP_], bf16)
            for r, srct, c in [(0, Ph, 0), (1, Ph, 0), (2, Pl, 0), (3, Pl, 0),
                               (4, Ph, 1), (5, Ph, 1), (6, Pl, 1), (7, Pl, 1)]:
                nc.gpsimd.dma_start(Plhs[r:r + 1, :, :], srct[:, c, :])
            nc.sync.dma_start(Plhs[8:11, :, :], ONESd[:])

            # ---------- main loop ----------
            partials = sb.tile([128, NI], f32)
            offs_all = sb.tile([128, NI * NT], i32)
            offs_f = sb.tile([128, NI * NT], f32)
            tidx_all = sb.tile([128, NI * NT, 8], u32)

            for b in range(NI):
                for t in range(NT):
                    s = ps.tile([128, NG], f32, tag="s")
                    for h in range(2):
                        nc.tensor.matmul(
                            s[:, h * 512:(h + 1) * 512],
                            Plhs[0:11, b, t * 128:(t + 1) * 128],
                            Grhs[0:11, b, h * 512:(h + 1) * 512],
                            start=True, stop=True,
                        )
                    top8 = sc.tile([128, 8], f32, tag="top8")
                    nc.vector.max(out=top8[:], in_=s[:])
                    nc.vector.max_index(
                        out=tidx_all[:, b * NT + t, :], in_max=top8[:], in_values=s[:]
                    )
                # offsets (global row in GTd) = tidx + 1024*b, via exact f32 arithmetic
                src = tidx_all[:, b * NT:(b + 1) * NT, 0]
                nc.vector.tensor_copy(offs_f[:, b * NT:(b + 1) * NT], src)
                nc.vector.tensor_scalar_add(
                    offs_f[:, b * NT:(b + 1) * NT],
                    offs_f[:, b * NT:(b + 1) * NT],
                    float(NG * b),
                )
            nc.vector.tensor_copy(offs_all[:], offs_f[:])

            for b in range(NI):
                gtm = sc.tile([128, NT, 2], f32, tag="gtm")
                for t in range(NT):
                    nc.gpsimd.indirect_dma_start(
                        out=gtm[:, t, :],
                        out_offset=None,
                        in_=GTd[:],
                        in_offset=bass.IndirectOffsetOnAxis(
                            ap=offs_all[:, b * NT + t:b * NT + t + 1], axis=0
                        ),
                    )
                pred = sc.tile([128, NT * 2], f32, tag="pred")
                nc.sync.dma_start(pred[:], PRd[b])
                diff = sc.tile([128, NT * 2], f32, tag="diff")
                nc.vector.tensor_sub(diff[:], pred[:], gtm[:].rearrange("p t c -> p (t c)"))
                nc.scalar.activation(out=diff[:], in_=diff[:], func=Abs,
                                     accum_out=partials[:, b:b + 1])

            col = sb.tile([128, 1], f32)
            nc.vector.reduce_sum(col[:], partials[:], axis=mybir.AxisListType.X)
            ones = sb.tile([128, 1], f32)
            nc.vector.memset(ones[:], 1.0)
            tot_ps = ps1.tile([1, 1], f32, tag="tot")
            nc.tensor.matmul(tot_ps[:], col[:], ones[:], start=True, stop=True)
            tot_sb = sb.tile([1, 1], f32)
            nc.scalar.copy(tot_sb[:], tot_ps[:])
            nc.sync.dma_start(LOSSd[:], tot_sb[:])
    return nc


_CACHED_NC = None


def _get_nc():
    global _CACHED_NC
    if _CACHED_NC is None:
        nc = bacc.Bacc("TRN2", target_bir_lowering=False, debug=False,
                       num_devices=NCORES)
        _build(nc)
        nc.finalize()
        _CACHED_NC = nc
    return _CACHED_NC


_QPERM = np.empty(NP_, dtype=np.int64)
for _t in range(NT):
    _QPERM[_t * 128:(_t + 1) * 128] = 4 * np.arange(128) + _t
_ONES = np.ones((3, NI, NP_), dtype=ml_dtypes.bfloat16)


def _make_in_maps(ini_pred_poly, pred_polys_, gt_polys):
    ini = np.ascontiguousarray(np.asarray(ini_pred_poly, dtype=np.float32))
    pred = np.ascontiguousarray(np.asarray(pred_polys_, dtype=np.float32))
    gt = np.ascontiguousarray(np.asarray(gt_polys, dtype=np.float32))
    in_maps = []
    for c in range(NCORES):
        sl = slice(c * NI, (c + 1) * NI)
        ini_c, pred_c, gt_c = ini[sl], pred[sl], gt[sl]
        P = np.empty((2, NI, NP_), dtype=np.float32)
        P[0] = ini_c[:, _QPERM, 0]
        P[1] = ini_c[:, _QPERM, 1]
        Gxy = np.ascontiguousarray(
            np.stack([gt_c[:, :, 0], gt_c[:, :, 1]]))
        PR = pred_c[:, _QPERM, :].reshape(NI, NT, 128, D).transpose(0, 2, 1, 3)
        PR = np.ascontiguousarray(PR.reshape(NI, 128, NT * D))
        in_maps.append({
            "Pd": P,
            "Gxyd": Gxy,
            "ONESd": _ONES,
            "GTd": np.ascontiguousarray(gt_c.reshape(NI * NG, D)),
            "PRd": PR,
        })
    return in_maps


def _run(in_maps, trace=False):
    nc = _get_nc()
    return bass_utils.run_bass_kernel_spmd(
        nc, in_maps, core_ids=list(range(NCORES)), trace=trace)


def kernel(ini_pred_poly, pred_polys_, gt_polys):
    in_maps = _make_in_maps(ini_pred_poly, pred_polys_, gt_polys)
    res = _run(in_maps)
    total = 0.0
    for c in range(NCORES):
        total += float(res.results[c]["LOSSd"][0, 0])
    return np.float32(total / (B * NP_ * D))


# revision 3
# speedup vs baseline: 1.4609x; 1.4609x over previous
"""Trainium2 Bass kernel for nn_MDLoss (retrieval_knn).

reference:
    distance[b, g, p] = ||ini_pred[b, p] - gt[b, g]||^2
    index_gt = argmin_g distance          -> [B, Np]
    gt_matched = gt[b, index_gt]          -> [B, Np, 2]
    loss = |pred - gt_matched|.mean()

Strategy (pure data-parallel over B across 8 cores, 32 instances each):
  - scores s[p, g] = 2*px*gx + 2*py*gy - (gx^2+gy^2); argmax_g s == argmin_g dist.
    Computed on the PE as a k=11 matmul of bf16 hi/lo-split operands (full PE
    rate, ~1e-6 absolute accuracy -> ~2e-6 relative error on the final mean).
       lhsT rows: [phx, phx, plx, plx, phy, phy, ply, ply, 1, 1, 1]
       rhs  rows: [Ghx, Glx, Ghx, Glx, Ghy, Gly, Ghy, Gly, R2h, R2m, R2l]
  - argmax per query via DVE max8 + max_index on the PSUM score tile.
  - gt gather via SWDGE indirect DMA (one offset per partition per call).
  - |pred - gt*| via DVE sub + ACT Abs with accumulate; partition reduce via
    a ones-matmul; per-core sum returned, combined on host in float64.

Layout: 512 queries/instance as 4 tiles of 128 partitions; position t*128+p of
the P rows holds query q = 4p+t, so per-tile argmax indices land in column t
of a [128, 4] offset tile matching a contiguous pred layout [128, (t, c)].
"""
import sys
import numpy as np

sys.path.insert(0, "/opt/trn_rl_repo")

import ml_dtypes  # noqa: E402
import concourse.bass as bass  # noqa: E402
import concourse.bacc as bacc  # noqa: E402
import concourse.tile as tile  # noqa: E402
from concourse import mybir  # noqa: E402
from concourse import bass_utils  # noqa: E402

B, NP_, NG, D = 256, 512, 1024, 2
NCORES = 8
NI = B // NCORES          # 32 instances per core
NT = NP_ // 128           # 4 query tiles per instance

f32 = mybir.dt.float32
bf16 = mybir.dt.bfloat16
u32 = mybir.dt.uint32
i32 = mybir.dt.int32
Sq = mybir.ActivationFunctionType.Square
Abs = mybir.ActivationFunctionType.Abs
SQH = 0.7071067811865476  # sqrt(1/2): Square(g*SQH) = g^2/2


def _build(nc):
    Pd = nc.dram_tensor("Pd", [2, NI, NP_], f32, kind="ExternalInput")
    Gxyd = nc.dram_tensor("Gxyd", [2, NI, NG], f32, kind="ExternalInput")
    ONESd = nc.dram_tensor("ONESd", [3, NI, NP_], bf16, kind="ExternalInput")
    GTd = nc.dram_tensor("GTd", [NI * NG, 2], f32, kind="ExternalInput")
    PRd = nc.dram_tensor("PRd", [NI, 128, NT * 2], f32, kind="ExternalInput")
    LOSSd = nc.dram_tensor("LOSSd", [1, 1], f32, kind="ExternalOutput")

    with tile.TileContext(nc) as tc:
        with (
            tc.tile_pool(name="sb", bufs=1) as sb,
            tc.tile_pool(name="sc", bufs=3) as sc,
            tc.tile_pool(name="ps", bufs=3, space="PSUM") as ps,
            tc.tile_pool(name="ps1", bufs=1, space="PSUM") as ps1,
        ):
            # ---------- G side: [2gx, 2gy, -(gx^2+gy^2)] + bf16 hi/lo split ----------
            Gsp = sb.tile([32, 3, NG], f32)
            nc.sync.dma_start(Gsp[:, 0:2, :], Gxyd[:].rearrange("c b g -> b c g"))
            Gsq = sb.tile([32, 2, NG], f32)
            nc.scalar.activation(Gsq[:], Gsp[:, 0:2, :], Sq, scale=SQH)  # g^2/2
            nc.vector.scalar_tensor_tensor(
                out=Gsp[:, 2, :], in0=Gsq[:, 0, :], scalar=-1.0, in1=Gsq[:, 1, :],
                op0=mybir.AluOpType.mult, op1=mybir.AluOpType.subtract,
            )  # -(gx^2+gy^2)/2
            nc.scalar.mul(Gsp[:], Gsp[:], 2.0)  # single final producer of Gsp

            Gh = sb.tile([32, 3, NG], bf16)
            nc.vector.tensor_copy(Gh[:], Gsp[:])
            T1 = sb.tile([32, 3, NG], f32)
            nc.vector.tensor_sub(T1[:], Gsp[:], Gh[:])
            Gl = sb.tile([32, 3, NG], bf16)
            nc.vector.tensor_copy(Gl[:], T1[:])
            R2l = sb.tile([32, NG], bf16)
            nc.vector.tensor_sub(R2l[:], T1[:, 2, :], Gl[:, 2, :])

            Grhs = sb.tile([11, NI, NG], bf16)
            for r, srct, c in [(0, Gh, 0), (1, Gl, 0), (2, Gh, 0), (3, Gl, 0),
                               (4, Gh, 1), (5, Gl, 1), (6, Gh, 1), (7, Gl, 1),
                               (8, Gh, 2), (9, Gl, 2)]:
                nc.gpsimd.dma_start(Grhs[r:r + 1, :, :], srct[:, c, :])
            nc.gpsimd.dma_start(Grhs[10:11, :, :], R2l[:, :])

            # ---------- P side: bf16 hi/lo split ----------
            Psp = sb.tile([32, 2, NP_], f32)
            nc.sync.dma_start(Psp[:], Pd[:].rearrange("c b q -> b c q"))
            Ph = sb.tile([32, 2, NP_], bf16)
            nc.vector.tensor_copy(Ph[:], Psp[:])
            Pl = sb.tile([32, 2, NP_], bf16)
            nc.vector.tensor_sub(Pl[:], Psp[:], Ph[:])

            Plhs = sb.tile([11, NI, NP_], bf16)
            for r, srct, c in [(0, Ph, 0), (1, Ph, 0), (2, Pl, 0), (3, Pl, 0),
                               (4, Ph, 1), (5, Ph, 1), (6, Pl, 1), (7, Pl, 1)]:
                nc.gpsimd.dma_start(Plhs[r:r + 1, :, :], srct[:, c, :])
            nc.sync.dma_start(Plhs[8:11, :, :], ONESd[:])

            # ---------- main loop ----------
            offs_all = sb.tile([128, NI * NT], i32)
            offs_f = sb.tile([128, NI * NT], f32)
            tidx_all = sb.tile([128, NI * NT, 8], u32)
            gtm_all = sb.tile([128, NI, NT, 2], f32)
            pred_all = sb.tile([128, NI, NT * 2], f32)
            nc.sync.dma_start(pred_all[:], PRd[:].rearrange("b p j -> p b j"))

            for b in range(NI):
                for t in range(NT):
                    s = ps.tile([128, NG], f32, tag="s")
                    for h in range(2):
                        nc.tensor.matmul(
                            s[:, h * 512:(h + 1) * 512],
                            Plhs[0:11, b, t * 128:(t + 1) * 128],
                            Grhs[0:11, b, h * 512:(h + 1) * 512],
                            start=True, stop=True,
                        )
                    top8 = sc.tile([128, 8], f32, tag="top8")
                    nc.vector.max(out=top8[:], in_=s[:])
                    nc.vector.max_index(
                        out=tidx_all[:, b * NT + t, :], in_max=top8[:], in_values=s[:]
                    )
                # offsets (global row in GTd) = tidx + 1024*b, via exact f32 arithmetic
                src = tidx_all[:, b * NT:(b + 1) * NT, 0]
                nc.vector.tensor_copy(offs_f[:, b * NT:(b + 1) * NT], src)
                nc.vector.tensor_scalar_add(
                    offs_f[:, b * NT:(b + 1) * NT],
                    offs_f[:, b * NT:(b + 1) * NT],
                    float(NG * b),
                )
                nc.vector.tensor_copy(offs_all[:, b * NT:(b + 1) * NT],
                                      offs_f[:, b * NT:(b + 1) * NT])
                for t in range(NT):
                    nc.gpsimd.indirect_dma_start(
                        out=gtm_all[:, b, t, :],
                        out_offset=None,
                        in_=GTd[:],
                        in_offset=bass.IndirectOffsetOnAxis(
                            ap=offs_all[:, b * NT + t:b * NT + t + 1], axis=0
                        ),
                    )

            diff = sb.tile([128, NI * NT * 2], f32)
            nc.vector.tensor_sub(diff[:], pred_all[:].rearrange("p b j -> p (b j)"),
                                 gtm_all[:].rearrange("p b t c -> p (b t c)"))
            col = sb.tile([128, 1], f32)
            nc.scalar.activation(out=diff[:], in_=diff[:], func=Abs,
                                 accum_out=col[:])
            ones = sb.tile([128, 1], f32)
            nc.vector.memset(ones[:], 1.0)
            tot_ps = ps1.tile([1, 1], f32, tag="tot")
            nc.tensor.matmul(tot_ps[:], col[:], ones[:], start=True, stop=True)
            tot_sb = sb.tile([1, 1], f32)
            nc.scalar.copy(tot_sb[:], tot_ps[:])
            nc.sync.dma_start(LOSSd[:], tot_sb[:])
    return nc


_CACHED_NC = None


def _get_nc():
    global _CACHED_NC
    if _CACHED_NC is None:
        nc = bacc.Bacc("TRN2", target_bir_lowering=False, debug=False,
                       num_devices=NCORES)
        _build(nc)
        nc.finalize()
        _CACHED_NC = nc
    return _CACHED_NC


_QPERM = np.empty(NP_, dtype=np.int64)
for _t in range(NT):
    _QPERM[_t * 128:(_t + 1) * 128] = 4 * np.arange(128) + _t
_ONES = np.ones((3, NI, NP_), dtype=ml_dtypes.bfloat16)


def _make_in_maps(ini_pred_poly, pred_polys_, gt_polys):
    ini = np.ascontiguousarray(np.asarray(ini_pred_poly, dtype=np.float32))
    pred = np.ascontiguousarray(np.asarray(pred_polys_, dtype=np.float32))
    gt = np.ascontiguousarray(np.asarray(gt_polys, dtype=np.float32))
    in_maps = []
    for c in range(NCORES):
        sl = slice(c * NI, (c + 1) * NI)
        ini_c, pred_c, gt_c = ini[sl], pred[sl], gt[sl]
        P = np.empty((2, NI, NP_), dtype=np.float32)
        P[0] = ini_c[:, _QPERM, 0]
        P[1] = ini_c[:, _QPERM, 1]
        Gxy = np.ascontiguousarray(
            np.stack([gt_c[:, :, 0], gt_c[:, :, 1]]))
        PR = pred_c[:, _QPERM, :].reshape(NI, NT, 128, D).transpose(0, 2, 1, 3)
        PR = np.ascontiguousarray(PR.reshape(NI, 128, NT * D))
        in_maps.append({
            "Pd": P,
            "Gxyd": Gxy,
            "ONESd": _ONES,
            "GTd": np.ascontiguousarray(gt_c.reshape(NI * NG, D)),
            "PRd": PR,
        })
    return in_maps


def _run(in_maps, trace=False):
    nc = _get_nc()
    return bass_utils.run_bass_kernel_spmd(
        nc, in_maps, core_ids=list(range(NCORES)), trace=trace)


def kernel(ini_pred_poly, pred_polys_, gt_polys):
    in_maps = _make_in_maps(ini_pred_poly, pred_polys_, gt_polys)
    res = _run(in_maps)
    total = 0.0
    for c in range(NCORES):
        total += float(res.results[c]["LOSSd"][0, 0])
    return np.float32(total / (B * NP_ * D))


# revision 4
# speedup vs baseline: 1.4617x; 1.0006x over previous
"""Trainium2 Bass kernel for nn_MDLoss (retrieval_knn).

reference:
    distance[b, g, p] = ||ini_pred[b, p] - gt[b, g]||^2
    index_gt = argmin_g distance          -> [B, Np]
    gt_matched = gt[b, index_gt]          -> [B, Np, 2]
    loss = |pred - gt_matched|.mean()

Strategy (pure data-parallel over B across 8 cores, 32 instances each):
  - scores s[p, g] = 2*px*gx + 2*py*gy - (gx^2+gy^2); argmax_g s == argmin_g dist.
    Computed on the PE as a k=11 matmul of bf16 hi/lo-split operands (full PE
    rate, ~1e-6 absolute accuracy -> ~2e-6 relative error on the final mean).
       lhsT rows: [phx, phx, plx, plx, phy, phy, ply, ply, 1, 1, 1]
       rhs  rows: [Ghx, Glx, Ghx, Glx, Ghy, Gly, Ghy, Gly, R2h, R2m, R2l]
  - argmax per query via DVE max8 + max_index on the PSUM score tile.
  - gt gather via SWDGE indirect DMA (one offset per partition per call).
  - |pred - gt*| via DVE sub + ACT Abs with accumulate; partition reduce via
    a ones-matmul; per-core sum returned, combined on host in float64.

Layout: 512 queries/instance as 4 tiles of 128 partitions; position t*128+p of
the P rows holds query q = 4p+t, so per-tile argmax indices land in column t
of a [128, 4] offset tile matching a contiguous pred layout [128, (t, c)].
"""
import sys
import numpy as np

sys.path.insert(0, "/opt/trn_rl_repo")

import ml_dtypes  # noqa: E402
import concourse.bass as bass  # noqa: E402
import concourse.bacc as bacc  # noqa: E402
import concourse.tile as tile  # noqa: E402
from concourse import mybir  # noqa: E402
from concourse import bass_utils  # noqa: E402

B, NP_, NG, D = 256, 512, 1024, 2
NCORES = 8
NI = B // NCORES          # 32 instances per core
NT = NP_ // 128           # 4 query tiles per instance

f32 = mybir.dt.float32
bf16 = mybir.dt.bfloat16
u32 = mybir.dt.uint32
i32 = mybir.dt.int32
Sq = mybir.ActivationFunctionType.Square
Abs = mybir.ActivationFunctionType.Abs
SQH = 0.7071067811865476  # sqrt(1/2): Square(g*SQH) = g^2/2


def _build(nc):
    Pd = nc.dram_tensor("Pd", [2, NI, NP_], f32, kind="ExternalInput")
    Gxyd = nc.dram_tensor("Gxyd", [2, NI, NG], f32, kind="ExternalInput")
    ONESd = nc.dram_tensor("ONESd", [3, NI, NP_], bf16, kind="ExternalInput")
    GTd = nc.dram_tensor("GTd", [NI * NG, 2], f32, kind="ExternalInput")
    PRd = nc.dram_tensor("PRd", [NI, 128, NT * 2], f32, kind="ExternalInput")
    LOSSd = nc.dram_tensor("LOSSd", [1, 1], f32, kind="ExternalOutput")

    with tile.TileContext(nc) as tc:
        with (
            tc.tile_pool(name="sb", bufs=1) as sb,
            tc.tile_pool(name="sc", bufs=3) as sc,
            tc.tile_pool(name="ps", bufs=3, space="PSUM") as ps,
            tc.tile_pool(name="ps1", bufs=1, space="PSUM") as ps1,
        ):
            # ---------- G side: [2gx, 2gy, -(gx^2+gy^2)] + bf16 hi/lo split ----------
            Gsp = sb.tile([32, 3, NG], f32)
            nc.sync.dma_start(Gsp[:, 0:2, :], Gxyd[:].rearrange("c b g -> b c g"))
            Gsq = sb.tile([32, 2, NG], f32)
            nc.scalar.activation(Gsq[:], Gsp[:, 0:2, :], Sq, scale=SQH)  # g^2/2
            nc.vector.scalar_tensor_tensor(
                out=Gsp[:, 2, :], in0=Gsq[:, 0, :], scalar=-1.0, in1=Gsq[:, 1, :],
                op0=mybir.AluOpType.mult, op1=mybir.AluOpType.subtract,
            )  # -(gx^2+gy^2)/2
            nc.scalar.mul(Gsp[:], Gsp[:], 2.0)  # single final producer of Gsp

            Gh = sb.tile([32, 3, NG], bf16)
            nc.vector.tensor_copy(Gh[:], Gsp[:])
            T1 = sb.tile([32, 3, NG], f32)
            nc.vector.tensor_sub(T1[:], Gsp[:], Gh[:])
            Gl = sb.tile([32, 3, NG], bf16)
            nc.vector.tensor_copy(Gl[:], T1[:])
            R2l = sb.tile([32, NG], bf16)
            nc.vector.tensor_sub(R2l[:], T1[:, 2, :], Gl[:, 2, :])

            Grhs = sb.tile([11, NI, NG], bf16)
            for r, srct, c in [(0, Gh, 0), (1, Gl, 0), (2, Gh, 0), (3, Gl, 0),
                               (4, Gh, 1), (5, Gl, 1), (6, Gh, 1), (7, Gl, 1),
                               (8, Gh, 2), (9, Gl, 2)]:
                nc.gpsimd.dma_start(Grhs[r:r + 1, :, :], srct[:, c, :])
            nc.gpsimd.dma_start(Grhs[10:11, :, :], R2l[:, :])

            # ---------- P side: bf16 hi/lo split ----------
            Psp = sb.tile([32, 2, NP_], f32)
            nc.sync.dma_start(Psp[:], Pd[:].rearrange("c b q -> b c q"))
            Ph = sb.tile([32, 2, NP_], bf16)
            nc.vector.tensor_copy(Ph[:], Psp[:])
            Pl = sb.tile([32, 2, NP_], bf16)
            nc.vector.tensor_sub(Pl[:], Psp[:], Ph[:])

            Plhs = sb.tile([11, NI, NP_], bf16)
            for r, srct, c in [(0, Ph, 0), (1, Ph, 0), (2, Pl, 0), (3, Pl, 0),
                               (4, Ph, 1), (5, Ph, 1), (6, Pl, 1), (7, Pl, 1)]:
                nc.gpsimd.dma_start(Plhs[r:r + 1, :, :], srct[:, c, :])
            nc.sync.dma_start(Plhs[8:11, :, :], ONESd[:])

            # ---------- main loop ----------
            gtm_all = sb.tile([128, NI, NT, 2], f32)
            pred_all = sb.tile([128, NI, NT * 2], f32)
            nc.sync.dma_start(pred_all[:], PRd[:].rearrange("b p j -> p b j"))

            for b in range(NI):
                tidx = sc.tile([128, NT, 8], u32, tag="tidx")
                for t in range(NT):
                    s = ps.tile([128, NG], f32, tag="s")
                    for h in range(2):
                        nc.tensor.matmul(
                            s[:, h * 512:(h + 1) * 512],
                            Plhs[0:11, b, t * 128:(t + 1) * 128],
                            Grhs[0:11, b, h * 512:(h + 1) * 512],
                            start=True, stop=True,
                        )
                    top8 = sc.tile([128, 8], f32, tag="top8")
                    nc.vector.max(out=top8[:], in_=s[:])
                    nc.vector.max_index(
                        out=tidx[:, t, :], in_max=top8[:], in_values=s[:]
                    )
                # offsets (global row in GTd) = tidx + 1024*b, via exact f32 arithmetic
                offs_f = sc.tile([128, NT], f32, tag="offs_f")
                offs_i = sc.tile([128, NT], i32, tag="offs_i")
                nc.vector.tensor_copy(offs_f[:], tidx[:, :, 0])
                nc.vector.tensor_scalar_add(offs_f[:], offs_f[:], float(NG * b))
                nc.vector.tensor_copy(offs_i[:], offs_f[:])
                for t in range(NT):
                    nc.gpsimd.indirect_dma_start(
                        out=gtm_all[:, b, t, :],
                        out_offset=None,
                        in_=GTd[:],
                        in_offset=bass.IndirectOffsetOnAxis(
                            ap=offs_i[:, t:t + 1], axis=0
                        ),
                    )

            diff = sb.tile([128, NI * NT * 2], f32)
            nc.vector.tensor_sub(diff[:], pred_all[:].rearrange("p b j -> p (b j)"),
                                 gtm_all[:].rearrange("p b t c -> p (b t c)"))
            col = sb.tile([128, 1], f32)
            nc.scalar.activation(out=diff[:], in_=diff[:], func=Abs,
                                 accum_out=col[:])
            ones = sb.tile([128, 1], f32)
            nc.vector.memset(ones[:], 1.0)
            tot_ps = ps1.tile([1, 1], f32, tag="tot")
            nc.tensor.matmul(tot_ps[:], col[:], ones[:], start=True, stop=True)
            tot_sb = sb.tile([1, 1], f32)
            nc.scalar.copy(tot_sb[:], tot_ps[:])
            nc.sync.dma_start(LOSSd[:], tot_sb[:])
    return nc


_CACHED_NC = None


def _get_nc():
    global _CACHED_NC
    if _CACHED_NC is None:
        nc = bacc.Bacc("TRN2", target_bir_lowering=False, debug=False,
                       num_devices=NCORES)
        _build(nc)
        nc.finalize()
        _CACHED_NC = nc
    return _CACHED_NC


_QPERM = np.empty(NP_, dtype=np.int64)
for _t in range(NT):
    _QPERM[_t * 128:(_t + 1) * 128] = 4 * np.arange(128) + _t
_ONES = np.ones((3, NI, NP_), dtype=ml_dtypes.bfloat16)


def _make_in_maps(ini_pred_poly, pred_polys_, gt_polys):
    ini = np.ascontiguousarray(np.asarray(ini_pred_poly, dtype=np.float32))
    pred = np.ascontiguousarray(np.asarray(pred_polys_, dtype=np.float32))
    gt = np.ascontiguousarray(np.asarray(gt_polys, dtype=np.float32))
    in_maps = []
    for c in range(NCORES):
        sl = slice(c * NI, (c + 1) * NI)
        ini_c, pred_c, gt_c = ini[sl], pred[sl], gt[sl]
        P = np.empty((2, NI, NP_), dtype=np.float32)
        P[0] = ini_c[:, _QPERM, 0]
        P[1] = ini_c[:, _QPERM, 1]
        Gxy = np.ascontiguousarray(
            np.stack([gt_c[:, :, 0], gt_c[:, :, 1]]))
        PR = pred_c[:, _QPERM, :].reshape(NI, NT, 128, D).transpose(0, 2, 1, 3)
        PR = np.ascontiguousarray(PR.reshape(NI, 128, NT * D))
        in_maps.append({
            "Pd": P,
            "Gxyd": Gxy,
            "ONESd": _ONES,
            "GTd": np.ascontiguousarray(gt_c.reshape(NI * NG, D)),
            "PRd": PR,
        })
    return in_maps


def _run(in_maps, trace=False):
    nc = _get_nc()
    return bass_utils.run_bass_kernel_spmd(
        nc, in_maps, core_ids=list(range(NCORES)), trace=trace)


def kernel(ini_pred_poly, pred_polys_, gt_polys):
    in_maps = _make_in_maps(ini_pred_poly, pred_polys_, gt_polys)
    res = _run(in_maps)
    total = 0.0
    for c in range(NCORES):
        total += float(res.results[c]["LOSSd"][0, 0])
    return np.float32(total / (B * NP_ * D))


# revision 5
# speedup vs baseline: 1.5334x; 1.0491x over previous
"""Trainium2 Bass kernel for nn_MDLoss (retrieval_knn).

reference:
    distance[b, g, p] = ||ini_pred[b, p] - gt[b, g]||^2
    index_gt = argmin_g distance          -> [B, Np]
    gt_matched = gt[b, index_gt]          -> [B, Np, 2]
    loss = |pred - gt_matched|.mean()

Strategy (pure data-parallel over B across 8 cores, 32 instances each):
  - scores s[p, g] = 2*px*gx + 2*py*gy - (gx^2+gy^2); argmax_g s == argmin_g dist.
    Computed on the PE as a k=11 matmul of bf16 hi/lo-split operands (full PE
    rate, ~1e-6 absolute accuracy -> ~2e-6 relative error on the final mean).
       lhsT rows: [phx, phx, plx, plx, phy, phy, ply, ply, 1, 1, 1]
       rhs  rows: [Ghx, Glx, Ghx, Glx, Ghy, Gly, Ghy, Gly, R2h, R2m, R2l]
  - argmax per query via DVE max8 + max_index on the PSUM score tile,
    processed in pairs of tiles so the DVE write-drain hides under the
    neighbor's op.
  - gt gather via SWDGE indirect DMA (one offset per partition per call),
    issued per instance so it overlaps the scan.
  - |pred - gt*| via one DVE sub + one ACT Abs with accumulate; partition
    reduce via a ones-matmul; per-core sum combined on host in float64.

Layout: 512 queries/instance as 4 tiles of 128 partitions; position t*128+p of
the P rows holds query q = 4p+t, so per-tile argmax indices land in column t
of a [128, 4] offset tile matching a contiguous pred layout [128, (t, c)].
"""
import sys
import numpy as np

sys.path.insert(0, "/opt/trn_rl_repo")

import ml_dtypes  # noqa: E402
import concourse.bass as bass  # noqa: E402
import concourse.bacc as bacc  # noqa: E402
import concourse.tile as tile  # noqa: E402
from concourse import mybir  # noqa: E402
from concourse import bass_utils  # noqa: E402

B, NP_, NG, D = 256, 512, 1024, 2
NCORES = 8
NI = B // NCORES          # 32 instances per core
NT = NP_ // 128           # 4 query tiles per instance

f32 = mybir.dt.float32
bf16 = mybir.dt.bfloat16
u32 = mybir.dt.uint32
i32 = mybir.dt.int32
Sq = mybir.ActivationFunctionType.Square
Abs = mybir.ActivationFunctionType.Abs
SQH = 0.7071067811865476  # sqrt(1/2): Square(g*SQH) = g^2/2


def _build(nc):
    Pd = nc.dram_tensor("Pd", [2, NI, NP_], f32, kind="ExternalInput")
    Gxyd = nc.dram_tensor("Gxyd", [2, NI, NG], f32, kind="ExternalInput")
    ONESd = nc.dram_tensor("ONESd", [3, NI, NP_], bf16, kind="ExternalInput")
    GTd = nc.dram_tensor("GTd", [NI * NG, 2], f32, kind="ExternalInput")
    PRd = nc.dram_tensor("PRd", [NI, 128, NT * 2], f32, kind="ExternalInput")
    LOSSd = nc.dram_tensor("LOSSd", [1, 1], f32, kind="ExternalOutput")

    with tile.TileContext(nc) as tc:
        with (
            tc.tile_pool(name="sb", bufs=1) as sb,
            tc.tile_pool(name="sc", bufs=3) as sc,
            tc.tile_pool(name="ps", bufs=3, space="PSUM") as ps,
            tc.tile_pool(name="ps1", bufs=1, space="PSUM") as ps1,
        ):
            # ---------- G/P build (chunked; separate tiles so the first
            # matmuls don't wait on the later chunks' assembly) ----------
            CHUNKS = [(0, 4), (4, NI)]
            Gtiles, Ptiles = [], []
            for ci, (lo, hi) in enumerate(CHUNKS):
                n = hi - lo
                bs = slice(lo, hi)
                Gsp = sb.tile([n, 3, NG], f32, tag=f"Gsp{ci}")
                nc.sync.dma_start(Gsp[:, 0:2, :],
                                  Gxyd[:, bs, :].rearrange("c b g -> b c g"))
                Gsq = sb.tile([n, 2, NG], f32, tag=f"Gsq{ci}")
                nc.scalar.activation(Gsq[:], Gsp[:, 0:2, :], Sq, scale=SQH)  # g^2/2
                nc.vector.scalar_tensor_tensor(
                    out=Gsp[:, 2, :], in0=Gsq[:, 0, :], scalar=-1.0,
                    in1=Gsq[:, 1, :],
                    op0=mybir.AluOpType.mult, op1=mybir.AluOpType.subtract,
                )  # -(gx^2+gy^2)/2
                nc.scalar.mul(Gsp[:], Gsp[:], 2.0)

                Gh = sb.tile([n, 3, NG], bf16, tag=f"Gh{ci}")
                nc.vector.tensor_copy(Gh[:], Gsp[:])
                T1 = sb.tile([n, 3, NG], f32, tag=f"T1{ci}")
                nc.vector.tensor_sub(T1[:], Gsp[:], Gh[:])
                Gl = sb.tile([n, 3, NG], bf16, tag=f"Gl{ci}")
                nc.vector.tensor_copy(Gl[:], T1[:])
                R2l = sb.tile([n, NG], bf16, tag=f"R2l{ci}")
                nc.vector.tensor_sub(R2l[:], T1[:, 2, :], Gl[:, 2, :])

                Grhs = sb.tile([11, n, NG], bf16, tag=f"Grhs{ci}")
                for r, srct, c in [(0, Gh, 0), (1, Gl, 0), (2, Gh, 0), (3, Gl, 0),
                                   (4, Gh, 1), (5, Gl, 1), (6, Gh, 1), (7, Gl, 1),
                                   (8, Gh, 2), (9, Gl, 2)]:
                    nc.gpsimd.dma_start(Grhs[r:r + 1, :, :], srct[:, c, :])
                nc.gpsimd.dma_start(Grhs[10:11, :, :], R2l[:, :])
                Gtiles.append(Grhs)

                Psp = sb.tile([n, 2, NP_], f32, tag=f"Psp{ci}")
                nc.sync.dma_start(Psp[:], Pd[:, bs, :].rearrange("c b q -> b c q"))
                Ph = sb.tile([n, 2, NP_], bf16, tag=f"Ph{ci}")
                nc.vector.tensor_copy(Ph[:], Psp[:])
                Pl = sb.tile([n, 2, NP_], bf16, tag=f"Pl{ci}")
                nc.vector.tensor_sub(Pl[:], Psp[:], Ph[:])

                Plhs = sb.tile([11, n, NP_], bf16, tag=f"Plhs{ci}")
                for r, srct, c in [(0, Ph, 0), (1, Ph, 0), (2, Pl, 0), (3, Pl, 0),
                                   (4, Ph, 1), (5, Ph, 1), (6, Pl, 1), (7, Pl, 1)]:
                    nc.gpsimd.dma_start(Plhs[r:r + 1, :, :], srct[:, c, :])
                nc.sync.dma_start(Plhs[8:11, :, :], ONESd[:, bs, :])
                Ptiles.append(Plhs)

            def tiles_of(b):
                for ci, (lo, hi) in enumerate(CHUNKS):
                    if lo <= b < hi:
                        return Ptiles[ci], Gtiles[ci], b - lo
                raise AssertionError

            # ---------- main loop ----------
            gtm_all = sb.tile([128, NI, NT, 2], f32)
            pred_all = sb.tile([128, NI, NT * 2], f32)
            nc.sync.dma_start(pred_all[:], PRd[:].rearrange("b p j -> p b j"))

            for b in range(NI):
                Plhs, Grhs, bl = tiles_of(b)
                tidx = sc.tile([128, NT, 8], u32, tag="tidx")
                for t0 in range(0, NT, 2):
                    pair = (t0, t0 + 1)
                    stiles, top8s = [], []
                    for t in pair:
                        s = ps.tile([128, NG], f32, tag="s")
                        for h in range(2):
                            nc.tensor.matmul(
                                s[:, h * 512:(h + 1) * 512],
                                Plhs[0:11, bl, t * 128:(t + 1) * 128],
                                Grhs[0:11, bl, h * 512:(h + 1) * 512],
                                start=True, stop=True,
                            )
                        stiles.append(s)
                    for t, s in zip(pair, stiles):
                        top8 = sc.tile([128, 8], f32, tag="top8")
                        nc.vector.max(out=top8[:], in_=s[:])
                        top8s.append(top8)
                    for t, s, top8 in zip(pair, stiles, top8s):
                        nc.vector.max_index(
                            out=tidx[:, t, :], in_max=top8[:], in_values=s[:]
                        )
                # offsets (global row in GTd) = tidx + 1024*b via exact f32 math
                offs_f = sc.tile([128, NT], f32, tag="offs_f")
                offs_i = sc.tile([128, NT], i32, tag="offs_i")
                nc.vector.tensor_copy(offs_f[:], tidx[:, :, 0])
                nc.vector.tensor_scalar_add(offs_f[:], offs_f[:], float(NG * b))
                nc.vector.tensor_copy(offs_i[:], offs_f[:])
                for t in range(NT):
                    nc.gpsimd.indirect_dma_start(
                        out=gtm_all[:, b, t, :],
                        out_offset=None,
                        in_=GTd[:],
                        in_offset=bass.IndirectOffsetOnAxis(
                            ap=offs_i[:, t:t + 1], axis=0
                        ),
                    )

            diff = sb.tile([128, NI * NT * 2], f32)
            nc.vector.tensor_sub(diff[:], pred_all[:].rearrange("p b j -> p (b j)"),
                                 gtm_all[:].rearrange("p b t c -> p (b t c)"))
            col = sb.tile([128, 1], f32)
            nc.scalar.activation(out=diff[:], in_=diff[:], func=Abs,
                                 accum_out=col[:])
            ones = sb.tile([128, 1], f32)
            nc.vector.memset(ones[:], 1.0)
            tot_ps = ps1.tile([1, 1], f32, tag="tot")
            nc.tensor.matmul(tot_ps[:], col[:], ones[:], start=True, stop=True)
            tot_sb = sb.tile([1, 1], f32)
            nc.scalar.copy(tot_sb[:], tot_ps[:])
            nc.sync.dma_start(LOSSd[:], tot_sb[:])
    return nc


_CACHED_NC = None


def _get_nc():
    global _CACHED_NC
    if _CACHED_NC is None:
        nc = bacc.Bacc("TRN2", target_bir_lowering=False, debug=False,
                       num_devices=NCORES)
        _build(nc)
        nc.finalize()
        _CACHED_NC = nc
    return _CACHED_NC


_QPERM = np.empty(NP_, dtype=np.int64)
for _t in range(NT):
    _QPERM[_t * 128:(_t + 1) * 128] = 4 * np.arange(128) + _t
_ONES = np.ones((3, NI, NP_), dtype=ml_dtypes.bfloat16)


def _make_in_maps(ini_pred_poly, pred_polys_, gt_polys):
    ini = np.ascontiguousarray(np.asarray(ini_pred_poly, dtype=np.float32))
    pred = np.ascontiguousarray(np.asarray(pred_polys_, dtype=np.float32))
    gt = np.ascontiguousarray(np.asarray(gt_polys, dtype=np.float32))
    in_maps = []
    for c in range(NCORES):
        sl = slice(c * NI, (c + 1) * NI)
        ini_c, pred_c, gt_c = ini[sl], pred[sl], gt[sl]
        P = np.empty((2, NI, NP_), dtype=np.float32)
        P[0] = ini_c[:, _QPERM, 0]
        P[1] = ini_c[:, _QPERM, 1]
        Gxy = np.ascontiguousarray(
            np.stack([gt_c[:, :, 0], gt_c[:, :, 1]]))
        PR = pred_c[:, _QPERM, :].reshape(NI, NT, 128, D).transpose(0, 2, 1, 3)
        PR = np.ascontiguousarray(PR.reshape(NI, 128, NT * D))
        in_maps.append({
            "Pd": P,
            "Gxyd": Gxy,
            "ONESd": _ONES,
            "GTd": np.ascontiguousarray(gt_c.reshape(NI * NG, D)),
            "PRd": PR,
        })
    return in_maps


def _run(in_maps, trace=False):
    nc = _get_nc()
    return bass_utils.run_bass_kernel_spmd(
        nc, in_maps, core_ids=list(range(NCORES)), trace=trace)


def kernel(ini_pred_poly, pred_polys_, gt_polys):
    in_maps = _make_in_maps(ini_pred_poly, pred_polys_, gt_polys)
    res = _run(in_maps)
    total = 0.0
    for c in range(NCORES):
        total += float(res.results[c]["LOSSd"][0, 0])
    return np.float32(total / (B * NP_ * D))
